# revision 1
# baseline (speedup 1.0000x reference)
"""BlockGRUCell fused Trainium2 kernel.

Sharding: data-parallel over batch across 8 NeuronCores (weights replicated).
Dataflow is fully transposed ([features, batch] on-chip): every matmul uses
the weight matrix in natural [in, out] layout as the stationary lhsT and the
transposed activations [in, batch] as the moving rhs, producing the next
layer's transposed activations directly. LayerNorm reductions run over the
partition axis via ones-vector matmuls; per-batch stats are broadcast back
across partitions with a rank-1 matmul.
"""
import numpy as np
from contextlib import ExitStack

import concourse.bass as bass
import concourse.tile as tile
from concourse import bacc, mybir
from concourse._compat import with_exitstack
from concourse.bass_utils import run_bass_kernel_spmd

B, D, S, A, H, G = 2048, 4096, 1024, 1024 // 8, 1024, 8
A = 128
DG = D // G            # 512
ING = DG + 3 * H       # 3584
NCORES = 8
BC = B // NCORES       # 256 batch rows per core
EPS = 1e-5
USE_SILU = True  # stage-1 only; CoreSim tests set False

F32 = mybir.dt.float32
F32R = mybir.dt.float32r
AF = mybir.ActivationFunctionType
OP = mybir.AluOpType

# vecs column layout (all per-partition tiled: col j holds v[j*128 + p])
NB_H = H // 128        # 8
NB_D = D // 128        # 32
NB_Z = 3 * D // 128    # 96
C_BD, C_GD, C_BED = 0, NB_H, 2 * NB_H
C_BS, C_GS, C_BES = 3 * NB_H, 4 * NB_H, 5 * NB_H
C_BA, C_GA, C_BEA = 6 * NB_H, 7 * NB_H, 8 * NB_H
C_DYNB, C_GDYN, C_BEDYN = 9 * NB_H, 9 * NB_H + NB_D, 9 * NB_H + 2 * NB_D
C_GRUB = 9 * NB_H + 3 * NB_D
NV = C_GRUB + NB_Z     # 264


@with_exitstack
def _emit(ctx: ExitStack, tc: tile.TileContext, ins: dict, outT: bass.AP):
    nc = tc.nc
    ctx.enter_context(nc.allow_low_precision(
        reason="float32r outputs are bit-identical fp32; needed as matmul operands"))

    persist = ctx.enter_context(tc.tile_pool(name="persist", bufs=1))
    sqp = ctx.enter_context(tc.tile_pool(name="sqp", bufs=2))
    small = ctx.enter_context(tc.tile_pool(name="small", bufs=1))
    consts = ctx.enter_context(tc.tile_pool(name="consts", bufs=1))
    mm_ps = ctx.enter_context(tc.tile_pool(name="mm_ps", bufs=1, space="PSUM"))
    st_ps = tc.alloc_tile_pool(name="st_ps", bufs=1, space="PSUM")
    bc_ps = tc.alloc_tile_pool(name="bc_ps", bufs=1, space="PSUM")
    wpool = tc.alloc_tile_pool(name="wpool", bufs=6)
    s1pool = tc.alloc_tile_pool(name="s1pool", bufs=1)
    s1upool = tc.alloc_tile_pool(name="s1upool", bufs=2)

    # ---- small constants ----
    vecs_sb = persist.tile([128, NV], F32, name="vecs")
    nc.sync.dma_start(out=vecs_sb, in_=ins["vecs"])
    ones_col_f = consts.tile([128, 1], F32)
    nc.vector.memset(ones_col_f, 1.0)
    ones_col = consts.tile([128, 1], F32R)
    nc.vector.tensor_copy(ones_col, ones_col_f)
    ones_row_f = consts.tile([1, 128], F32)
    nc.vector.memset(ones_row_f, 1.0)
    ones_row = consts.tile([1, 128], F32R)
    nc.vector.tensor_copy(ones_row, ones_row_f)
    eps_t = consts.tile([1, 1], F32)
    nc.vector.memset(eps_t, EPS)

    x_sb = persist.tile([128, 24, BC], F32R, name="x")

    mm_tags = [f"up{i}" for i in range(4)]

    def mm_tile(i):
        return mm_ps.tile([128, 2 * BC], F32, name=mm_tags[i % 4])[:, :BC]

    z_state = {}

    def z_pair(i):
        tags = [(mm_ps, t) for t in mm_tags]
        if "z2" in z_state:
            tags += [(z_state["z2"], f"z2_{j}") for j in range(4)]
        pool, t = tags[i % len(tags)]
        return pool.tile([128, 2 * BC], F32, name=t)

    def stats_finish(ssum, ssq, nfeat):
        """Broadcast LN stats: returns (meanB2, rstdB2) [128, 2*BC] SBUF tiles
        with the per-batch stat duplicated in both halves (for paired apply)."""
        mr2 = small.tile([1, 4 * BC], F32R, name="mr2")
        mean, mean_b = mr2[:, :BC], mr2[:, BC:2 * BC]
        rstd, rstd_b = mr2[:, 2 * BC:3 * BC], mr2[:, 3 * BC:]
        nc.vector.tensor_scalar_mul(mean, ssum, 1.0 / nfeat)
        nc.vector.tensor_copy(mean_b, mean)
        m2 = small.tile([1, BC], F32, name="m2")
        nc.vector.tensor_mul(m2, mean, mean)
        var = small.tile([1, BC], F32, name="var")
        nc.vector.tensor_scalar(var, ssq, 1.0 / nfeat, None, OP.mult, OP.bypass)
        nc.vector.tensor_sub(var, var, m2)
        std = small.tile([1, BC], F32, name="std")
        nc.scalar.activation(std, var, AF.Sqrt, bias=eps_t, scale=1.0)
        nc.vector.reciprocal(rstd, std)
        nc.vector.tensor_copy(rstd_b, rstd)
        bc0 = bc_ps.tile([128, 2 * BC], F32, name="bc0")
        nc.tensor.matmul(bc0, ones_row, mr2[:, :2 * BC], start=True, stop=True)
        bc1 = bc_ps.tile([128, 2 * BC], F32, name="bc1")
        nc.tensor.matmul(bc1, ones_row, mr2[:, 2 * BC:], start=True, stop=True)
        mr0 = sqp.tile([128, 2 * BC], F32, name="mr0")
        nc.vector.tensor_copy(mr0, bc0)
        mr1 = sqp.tile([128, 2 * BC], F32, name="mr1")
        nc.vector.tensor_copy(mr1, bc1)
        return mr0, mr1

    def ln_apply(u_sb_pair, meanB2, rstdB2, g_cols, be_cols, dst_fn, pair_list):
        """Paired LN apply: dst = silu(((u - m) * r) * gamma + beta).

        u_sb_pair(i) -> [128, 2*BC] view of feature tiles (i, i+1).
        Sub/mul run on [128, 512] pairs; the per-tile gamma/beta + silu run on
        the ACT LUT per half (one function -> one table load)."""
        for i in pair_list:
            tp = sqp.tile([128, 2 * BC], F32, name="tp")
            nc.vector.tensor_sub(tp, u_sb_pair(i), meanB2)
            nc.vector.tensor_mul(tp, tp, rstdB2)
            for h in (0, 1):
                idx = i + h
                gcol = vecs_sb[:, g_cols + idx:g_cols + idx + 1]
                becol = vecs_sb[:, be_cols + idx:be_cols + idx + 1]
                th = tp[:, h * BC:(h + 1) * BC]
                if USE_SILU:
                    nc.scalar.activation(dst_fn(idx), th, AF.Silu,
                                         bias=becol, scale=gcol)
                else:
                    sg = sqp.tile([128, BC], F32, name="sg")
                    nc.scalar.activation(sg, th, AF.Sigmoid, bias=becol, scale=gcol)
                    nn = sqp.tile([128, BC], F32, name="nn")
                    nc.gpsimd.tensor_scalar(nn, th, gcol, becol, OP.mult, OP.add)
                    nc.gpsimd.tensor_mul(dst_fn(idx), nn, sg)

    # ================= stage 1: three input projections =================
    deterT_sb = persist.tile([128, NB_D, BC], F32R, name="deterT")
    s1_rhs = {}

    def load_s1_rhs(wname):
        # emitted per-tensor so act loads sit just ahead of their weight
        # panels in the HWDGE FIFO (no head-of-line block of first matmuls)
        if wname == "W_a":
            t = s1pool.tile([128, 1, BC], F32R, name="actionT")
            nc.sync.dma_start(out=t, in_=ins["actionT"].bitcast(F32R)
                              .rearrange("(t p) b -> p t b", p=128))
        elif wname == "W_s":
            t = s1pool.tile([128, S // 128, BC], F32R, name="stochT")
            nc.sync.dma_start(out=t, in_=ins["stochT"].bitcast(F32R)
                              .rearrange("(t p) b -> p t b", p=128))
        else:
            t = deterT_sb
            _dT = ins["deterT"].bitcast(F32R).rearrange("(t p) b -> p t b", p=128)
            for q in range(4):
                nc.sync.dma_start(out=t[:, q * 8:(q + 1) * 8, :],
                                  in_=_dT[:, q * 8:(q + 1) * 8, :])
        return t

    stage1 = [
        ("W_a", 1, C_BA, C_GA, C_BEA, 16),
        ("W_s", S // 128, C_BS, C_GS, C_BES, 8),
        ("W_d", NB_D, C_BD, C_GD, C_BED, 0),
    ]
    for wname, KT, bcol, gcol, becol, xoff in stage1:
        Wap = ins[wname]
        rhs_sb = load_s1_rhs(wname)
        u_sb = s1upool.tile([128, NB_H, BC], F32R, name="u_sb")
        ssum = st_ps.tile([1, BC], F32, name="ssum")
        ssq = st_ps.tile([1, BC], F32, name="ssq")
        for c in range(2):  # H in two chunks of 4 m-tiles
            psums = [mm_tile(m) for m in range(4)]
            KG = (KT + 3) // 4
            for kg in range(KG):
                nk = min(4, KT - kg * 4)
                wp = wpool.tile([128, 4, 512], F32R, name="wp")
                nc.sync.dma_start(
                    out=wp[:, :nk, :],
                    in_=Wap.bitcast(F32R)[kg * 512:kg * 512 + nk * 128,
                                          c * 512:(c + 1) * 512]
                    .rearrange("(kk p) n -> p kk n", p=128),
                )
                for k4 in range(nk):
                    k = kg * 4 + k4
                    for m in range(4):
                        nc.tensor.matmul(
                            psums[m],
                            wp[:, k4, m * 128:(m + 1) * 128],
                            rhs_sb[:, k, :],
                            start=(k == 0), stop=(k == KT - 1),
                        )
            for m in range(4):
                mt = c * 4 + m
                ut = u_sb[:, mt, :]
                nc.vector.tensor_scalar_add(ut, psums[m],
                                            vecs_sb[:, bcol + mt:bcol + mt + 1])
                usq = sqp.tile([128, BC], F32R, name="usq")
                nc.vector.tensor_mul(usq, ut, ut)
                nc.tensor.matmul(ssum, ones_col, ut,
                                 start=(mt == 0), stop=(mt == NB_H - 1))
                nc.tensor.matmul(ssq, ones_col, usq,
                                 start=(mt == 0), stop=(mt == NB_H - 1))
        meanB2, rstdB2 = stats_finish(ssum, ssq, H)
        ln_apply(lambda i, u=u_sb: u[:, i:i + 2, :].rearrange("p a b -> p (a b)"),
                 meanB2, rstdB2, gcol, becol,
                 lambda idx, xoff=xoff: x_sb[:, xoff + idx, :],
                 [0, 2, 4, 6])
    s1upool.release()
    s1pool.release()

    # ================= stage 2: block-diagonal dyn layer =================
    dynW = ins["dyn_W"]  # [G, ING, DG]
    y_sb = persist.tile([128, NB_D, BC], F32R, name="y_sb")
    yssum = st_ps.tile([1, BC], F32, name="ssum")
    yssq = st_ps.tile([1, BC], F32, name="ssq")
    KT2 = ING // 128  # 28

    def rhs2(g, k):
        return deterT_sb[:, g * 4 + k, :] if k < 4 else x_sb[:, k - 4, :]

    for g in range(G):
        psums = [mm_tile(m) for m in range(4)]
        for kg in range(7):
            wp = wpool.tile([128, 4, 512], F32R, name="wp")
            nc.sync.dma_start(
                out=wp,
                in_=dynW.bitcast(F32R)[g, kg * 512:(kg + 1) * 512, :]
                .rearrange("(kk p) n -> p kk n", p=128),
            )
            for k4 in range(4):
                k = kg * 4 + k4
                for m in range(4):
                    nc.tensor.matmul(
                        psums[m],
                        wp[:, k4, m * 128:(m + 1) * 128],
                        rhs2(g, k),
                        start=(k == 0), stop=(k == KT2 - 1),
                    )
        for m in range(4):
            ft = g * 4 + m
            yt = y_sb[:, ft, :]
            nc.vector.tensor_scalar_add(yt, psums[m],
                                        vecs_sb[:, C_DYNB + ft:C_DYNB + ft + 1])
            ysq = sqp.tile([128, BC], F32R, name="usq")
            nc.vector.tensor_mul(ysq, yt, yt)
            nc.tensor.matmul(yssum, ones_col, yt,
                             start=(ft == 0), stop=(ft == NB_D - 1))
            nc.tensor.matmul(yssq, ones_col, ysq,
                             start=(ft == 0), stop=(ft == NB_D - 1))
    meanB2, rstdB2 = stats_finish(yssum, yssq, D)
    # bulk apply in stage-3 first-use order; Silu only -> one ACT table load
    _border = [0, 2, 5, 3, 6, 1, 4, 7]
    ln_apply(lambda i: y_sb[:, i:i + 2, :].rearrange("p a b -> p (a b)"),
             meanB2, rstdB2, C_GDYN, C_BEDYN, lambda idx: y_sb[:, idx, :],
             [g * 4 + j for g in _border for j in (0, 2)])
    bc_ps.release()
    st_ps.release()
    z2_ps = tc.alloc_tile_pool(name="z2_ps", bufs=1, space="PSUM")
    z_state["z2"] = z2_ps

    # ================= stage 3: GRU gates + output =================
    # zflat f-tiles: reset tj, cand tj+32, update tj+64; block gf = f*128//1536.
    gruW = ins["gru_W"]  # [G, DG, 3*DG]
    zcnt = 0
    for seg in range(8):  # 4 tj per segment; each offset's 4 f-tiles in one block
        tj0 = seg * 4
        tjs = list(range(tj0, tj0 + 4))
        panels = []
        for off in range(3):
            f0 = tj0 * 128 + off * 4096
            gf, col0 = f0 // 1536, f0 % 1536
            wp = wpool.tile([128, 4, 512], F32R, name="wp")
            nc.sync.dma_start(
                out=wp,
                in_=gruW.bitcast(F32R)[gf, :, col0:col0 + 512]
                .rearrange("(kk p) n -> p kk n", p=128),
            )
            panels.append((wp, gf))

        def zmm(off, tj):
            nonlocal zcnt
            co = (tj - tj0) * 128
            wp, gf = panels[off]
            zp = z_pair(zcnt)[:, :BC]
            zcnt += 1
            for k in range(4):
                nc.tensor.matmul(zp, wp[:, k, co:co + 128], y_sb[:, gf * 4 + k, :],
                                 start=(k == 0), stop=(k == 3))
            return zp

        r, uu, rc = {}, {}, {}
        for tj in tjs:  # reset gate: sigmoid batch
            zp = zmm(0, tj)
            r[tj] = sqp.tile([128, BC], F32, name=f"r{tj % 2}")
            nc.scalar.activation(r[tj], zp, AF.Sigmoid,
                                 bias=vecs_sb[:, C_GRUB + tj:C_GRUB + tj + 1], scale=1.0)
        for tj in tjs:  # update gate: sigmoid batch
            zp = zmm(2, tj)
            uu[tj] = sqp.tile([128, BC], F32, name=f"uu{tj % 2}")
            nc.scalar.activation(uu[tj], zp, AF.Sigmoid,
                                 bias=vecs_sb[:, C_GRUB + 64 + tj:C_GRUB + 64 + tj + 1], scale=1.0)
        for tj in tjs:  # cand pre-act: DVE + gpsimd only
            zp = zmm(1, tj)
            cp = sqp.tile([128, BC], F32, name="cp")
            nc.vector.tensor_scalar_add(cp, zp,
                                        vecs_sb[:, C_GRUB + 32 + tj:C_GRUB + 32 + tj + 1])
            rc[tj] = sqp.tile([128, BC], F32, name=f"rc{tj % 2}")
            nc.gpsimd.tensor_mul(rc[tj], r[tj], cp)
        for tj in tjs:  # tanh via 2*sigmoid(2x)-1; out = uu*(c-d)+d
            ss = sqp.tile([128, BC], F32, name="cc")
            nc.scalar.activation(ss, rc[tj], AF.Sigmoid, bias=0.0, scale=2.0)
            dt_ = deterT_sb[:, tj, :]
            cd = sqp.tile([128, BC], F32, name="cd")
            nc.vector.scalar_tensor_tensor(cd, ss, 2.0, dt_, OP.mult, OP.subtract)
            o = sqp.tile([128, BC], F32, name="o")
            nc.vector.scalar_tensor_tensor(o, cd, 1.0, uu[tj], OP.subtract, OP.mult)
            nc.gpsimd.tensor_add(o, o, dt_)
            nc.sync.dma_start(out=outT[tj * 128:(tj + 1) * 128, :], in_=o)
    z2_ps.release()
    wpool.release()


_CACHE = {}


def _build():
    if "nc" in _CACHE:
        return _CACHE["nc"]
    nc = bacc.Bacc("TRN2", target_bir_lowering=False, debug=False,
                   num_devices=NCORES)
    ins = {}
    for name, shape in [
        ("deterT", [D, BC]), ("stochT", [S, BC]), ("actionT", [A, BC]),
        ("W_d", [D, H]), ("W_s", [S, H]), ("W_a", [A, H]),
        ("dyn_W", [G, ING, DG]), ("gru_W", [G, DG, 3 * DG]),
        ("vecs", [128, NV]),
    ]:
        ins[name] = nc.dram_tensor(name, shape, F32, kind="ExternalInput").ap()
    outT = nc.dram_tensor("outT", [D, BC], F32, kind="ExternalOutput").ap()
    with tile.TileContext(nc) as tc:
        _emit(tc, ins, outT)
    nc.compile()
    _CACHE["nc"] = nc
    return nc


def _col_tile(v):
    """[L] -> [128, L//128] with col t holding v[t*128 + p]."""
    return np.ascontiguousarray(v.reshape(-1, 128).T.astype(np.float32))


def _make_vecs(b_d, g_d, be_d, b_s, g_s, be_s, b_a, g_a, be_a,
               dyn_b, g_dyn, be_dyn, gru_b):
    gru_adj = np.array(gru_b, dtype=np.float32).copy()
    gru_adj[2 * D:] -= 1.0
    cols = [b_d, g_d, be_d, b_s, g_s, be_s, b_a, g_a, be_a,
            dyn_b, g_dyn, be_dyn, gru_adj]
    return np.concatenate([_col_tile(np.asarray(c)) for c in cols], axis=1), gru_adj


def kernel(deter, stoch, action,
           W_d, b_d, g_d, be_d,
           W_s, b_s, g_s, be_s,
           W_a, b_a, g_a, be_a,
           dyn_W, dyn_b, g_dyn, be_dyn,
           gru_W, gru_b):
    nc = _build()

    deterT = np.ascontiguousarray(np.asarray(deter, dtype=np.float32).T)
    stochT = np.ascontiguousarray(np.asarray(stoch, dtype=np.float32).T)
    actionT = np.ascontiguousarray(np.asarray(action, dtype=np.float32).T)
    vecs, gru_adj = _make_vecs(b_d, g_d, be_d, b_s, g_s, be_s, b_a, g_a, be_a,
                               dyn_b, g_dyn, be_dyn, gru_b)
    shared = {
        "W_d": np.ascontiguousarray(np.asarray(W_d, dtype=np.float32)),
        "W_s": np.ascontiguousarray(np.asarray(W_s, dtype=np.float32)),
        "W_a": np.ascontiguousarray(np.asarray(W_a, dtype=np.float32)),
        "dyn_W": np.ascontiguousarray(np.asarray(dyn_W, dtype=np.float32)),
        "gru_W": np.ascontiguousarray(np.asarray(gru_W, dtype=np.float32)),
        "vecs": vecs,
    }
    in_maps = []
    for c in range(NCORES):
        sl = slice(c * BC, (c + 1) * BC)
        m = dict(shared)
        m["deterT"] = np.ascontiguousarray(deterT[:, sl])
        m["stochT"] = np.ascontiguousarray(stochT[:, sl])
        m["actionT"] = np.ascontiguousarray(actionT[:, sl])
        in_maps.append(m)

    import os
    kw = {}
    if os.environ.get("BASS_TMPDIR"):
        kw["tmpdir"] = os.environ["BASS_TMPDIR"]
    res = run_bass_kernel_spmd(nc, in_maps, list(range(NCORES)), **kw)
    global LAST_RES
    LAST_RES = res
    outT = np.concatenate([res.results[c]["outT"] for c in range(NCORES)], axis=1)
    return np.ascontiguousarray(outT.T)


LAST_RES = None



# revision 16
# speedup vs baseline: 1.1805x; 1.1805x over previous
"""BlockGRUCell fused Trainium2 kernel.

Sharding: data-parallel over batch across 8 NeuronCores (weights replicated).
Dataflow is fully transposed ([features, batch] on-chip): every matmul uses
the weight matrix in natural [in, out] layout as the stationary lhsT and the
transposed activations [in, batch] as the moving rhs, producing the next
layer's transposed activations directly. LayerNorm reductions run over the
partition axis via ones-vector matmuls; per-batch stats are broadcast back
across partitions with a rank-1 matmul.
"""
import numpy as np
from contextlib import ExitStack

import concourse.bass as bass
import concourse.tile as tile
from concourse import bacc, mybir
from concourse._compat import with_exitstack
from concourse.bass_utils import run_bass_kernel_spmd

B, D, S, A, H, G = 2048, 4096, 1024, 1024 // 8, 1024, 8
A = 128
DG = D // G            # 512
ING = DG + 3 * H       # 3584
NCORES = 8
BC = B // NCORES       # 256 batch rows per core
EPS = 1e-5
USE_SILU = True  # stage-1 only; CoreSim tests set False

F32 = mybir.dt.float32
F32R = mybir.dt.float32r
BF16 = mybir.dt.bfloat16
AF = mybir.ActivationFunctionType
OP = mybir.AluOpType

# vecs column layout (all per-partition tiled: col j holds v[j*128 + p])
NB_H = H // 128        # 8
NB_D = D // 128        # 32
NB_Z = 3 * D // 128    # 96
C_BD, C_GD, C_BED = 0, NB_H, 2 * NB_H
C_BS, C_GS, C_BES = 3 * NB_H, 4 * NB_H, 5 * NB_H
C_BA, C_GA, C_BEA = 6 * NB_H, 7 * NB_H, 8 * NB_H
C_DYNB, C_GDYN, C_BEDYN = 9 * NB_H, 9 * NB_H + NB_D, 9 * NB_H + 2 * NB_D
C_GRUB = 9 * NB_H + 3 * NB_D
NV = C_GRUB + NB_Z     # 264


@with_exitstack
def _emit(ctx: ExitStack, tc: tile.TileContext, ins: dict, outT: bass.AP):
    nc = tc.nc
    ctx.enter_context(nc.allow_low_precision(
        reason="float32r outputs are bit-identical fp32; needed as matmul operands"))

    persist = ctx.enter_context(tc.tile_pool(name="persist", bufs=1))
    sqp = ctx.enter_context(tc.tile_pool(name="sqp", bufs=2))
    small = ctx.enter_context(tc.tile_pool(name="small", bufs=1))
    consts = ctx.enter_context(tc.tile_pool(name="consts", bufs=1))
    mm_ps = ctx.enter_context(tc.tile_pool(name="mm_ps", bufs=1, space="PSUM"))
    st_ps = tc.alloc_tile_pool(name="st_ps", bufs=1, space="PSUM")
    bc_ps = tc.alloc_tile_pool(name="bc_ps", bufs=1, space="PSUM")
    wpool = tc.alloc_tile_pool(name="wpool", bufs=6)
    s1pool = tc.alloc_tile_pool(name="s1pool", bufs=1)
    s1upool = tc.alloc_tile_pool(name="s1upool", bufs=2)

    # ---- small constants ----
    vecs_sb = persist.tile([128, NV], F32, name="vecs")
    nc.sync.dma_start(out=vecs_sb, in_=ins["vecs"])
    ones_col_f = consts.tile([128, 1], F32)
    nc.vector.memset(ones_col_f, 1.0)
    ones_col = consts.tile([128, 1], BF16)
    nc.vector.tensor_copy(ones_col, ones_col_f)
    ones_row_f = consts.tile([1, 128], F32)
    nc.vector.memset(ones_row_f, 1.0)
    ones_row = consts.tile([1, 128], F32R)
    nc.vector.tensor_copy(ones_row, ones_row_f)
    eps_t = consts.tile([1, 1], F32)
    nc.vector.memset(eps_t, EPS)

    x_sb = persist.tile([128, 24, BC], BF16, name="x")

    mm_tags = [f"up{i}" for i in range(4)]

    def mm_tile(i):
        return mm_ps.tile([128, 2 * BC], F32, name=mm_tags[i % 4])[:, :BC]

    z_state = {}

    def z_pair(i):
        tags = [(mm_ps, t) for t in mm_tags]
        if "z2" in z_state:
            tags += [(z_state["z2"], f"z2_{j}") for j in range(4)]
        pool, t = tags[i % len(tags)]
        return pool.tile([128, 2 * BC], F32, name=t)

    def stats_finish(ssum, ssq, nfeat):
        """Broadcast LN stats: returns (meanB2, rstdB2) [128, 2*BC] SBUF tiles
        with the per-batch stat duplicated in both halves (for paired apply)."""
        mr2 = small.tile([1, 4 * BC], F32R, name="mr2")
        mean, mean_b = mr2[:, :BC], mr2[:, BC:2 * BC]
        rstd, rstd_b = mr2[:, 2 * BC:3 * BC], mr2[:, 3 * BC:]
        nc.vector.tensor_scalar_mul(mean, ssum, 1.0 / nfeat)
        nc.vector.tensor_copy(mean_b, mean)
        m2 = small.tile([1, BC], F32, name="m2")
        nc.vector.tensor_mul(m2, mean, mean)
        var = small.tile([1, BC], F32, name="var")
        nc.vector.tensor_scalar(var, ssq, 1.0 / nfeat, None, OP.mult, OP.bypass)
        nc.vector.tensor_sub(var, var, m2)
        std = small.tile([1, BC], F32, name="std")
        nc.scalar.activation(std, var, AF.Sqrt, bias=eps_t, scale=1.0)
        nc.vector.reciprocal(rstd, std)
        nc.vector.tensor_copy(rstd_b, rstd)
        bc0 = bc_ps.tile([128, 2 * BC], F32, name="bc0")
        nc.tensor.matmul(bc0, ones_row, mr2[:, :2 * BC], start=True, stop=True)
        bc1 = bc_ps.tile([128, 2 * BC], F32, name="bc1")
        nc.tensor.matmul(bc1, ones_row, mr2[:, 2 * BC:], start=True, stop=True)
        mr0 = sqp.tile([128, 2 * BC], F32, name="mr0")
        nc.vector.tensor_copy(mr0, bc0)
        mr1 = sqp.tile([128, 2 * BC], F32, name="mr1")
        nc.vector.tensor_copy(mr1, bc1)
        return mr0, mr1

    def ln_apply(u_sb_pair, meanB2, rstdB2, g_cols, be_cols, dst_fn, pair_list):
        """Paired LN apply: dst = silu(((u - m) * r) * gamma + beta).

        u_sb_pair(i) -> [128, 2*BC] view of feature tiles (i, i+1).
        Sub/mul run on [128, 512] pairs; the per-tile gamma/beta + silu run on
        the ACT LUT per half (one function -> one table load)."""
        for i in pair_list:
            tp = sqp.tile([128, 2 * BC], F32, name="tp")
            nc.vector.tensor_sub(tp, u_sb_pair(i), meanB2)
            nc.vector.tensor_mul(tp, tp, rstdB2)
            for h in (0, 1):
                idx = i + h
                gcol = vecs_sb[:, g_cols + idx:g_cols + idx + 1]
                becol = vecs_sb[:, be_cols + idx:be_cols + idx + 1]
                th = tp[:, h * BC:(h + 1) * BC]
                if USE_SILU:
                    nc.scalar.activation(dst_fn(idx), th, AF.Silu,
                                         bias=becol, scale=gcol)
                else:
                    sg = sqp.tile([128, BC], F32, name="sg")
                    nc.scalar.activation(sg, th, AF.Sigmoid, bias=becol, scale=gcol)
                    nn = sqp.tile([128, BC], F32, name="nn")
                    nc.gpsimd.tensor_scalar(nn, th, gcol, becol, OP.mult, OP.add)
                    nc.gpsimd.tensor_mul(dst_fn(idx), nn, sg)

    # ================= stage 1: three input projections =================
    deterT_sb = persist.tile([128, NB_D, BC], BF16, name="deterT")
    s1_rhs = {}

    def load_s1_rhs(wname):
        # emitted per-tensor so act loads sit just ahead of their weight
        # panels in the HWDGE FIFO (no head-of-line block of first matmuls)
        if wname == "W_a":
            t = s1pool.tile([128, 1, BC], BF16, name="actionT")
            nc.sync.dma_start(out=t, in_=ins["actionT"]
                              .rearrange("(t p) b -> p t b", p=128))
        elif wname == "W_s":
            t = s1pool.tile([128, S // 128, BC], BF16, name="stochT")
            nc.sync.dma_start(out=t, in_=ins["stochT"]
                              .rearrange("(t p) b -> p t b", p=128))
        else:
            t = deterT_sb
            _dT = ins["deterT"].rearrange("(t p) b -> p t b", p=128)
            for q in range(4):
                nc.sync.dma_start(out=t[:, q * 8:(q + 1) * 8, :],
                                  in_=_dT[:, q * 8:(q + 1) * 8, :])
        return t

    stage1 = [
        ("W_a", 1, C_BA, C_GA, C_BEA, 16),
        ("W_s", S // 128, C_BS, C_GS, C_BES, 8),
        ("W_d", NB_D, C_BD, C_GD, C_BED, 0),
    ]
    for wname, KT, bcol, gcol, becol, xoff in stage1:
        Wap = ins[wname]
        rhs_sb = load_s1_rhs(wname)
        u_sb = s1upool.tile([128, NB_H, BC], BF16, name="u_sb")
        ssum = st_ps.tile([1, BC], F32, name="ssum")
        ssq = st_ps.tile([1, BC], F32, name="ssq")
        for c in range(2):  # H in two chunks of 4 m-tiles
            psums = [mm_tile(m) for m in range(4)]
            KG = (KT + 3) // 4
            for kg in range(KG):
                nk = min(4, KT - kg * 4)
                wp = wpool.tile([128, 4, 512], BF16, name="wp")
                nc.sync.dma_start(
                    out=wp[:, :nk, :],
                    in_=Wap[kg * 512:kg * 512 + nk * 128,
                            c * 512:(c + 1) * 512]
                    .rearrange("(kk p) n -> p kk n", p=128),
                )
                for k4 in range(nk):
                    k = kg * 4 + k4
                    for m in range(4):
                        nc.tensor.matmul(
                            psums[m],
                            wp[:, k4, m * 128:(m + 1) * 128],
                            rhs_sb[:, k, :],
                            start=(k == 0), stop=(k == KT - 1),
                        )
            for m in range(4):
                mt = c * 4 + m
                ut = u_sb[:, mt, :]
                nc.vector.tensor_scalar_add(ut, psums[m],
                                            vecs_sb[:, bcol + mt:bcol + mt + 1])
                usq = sqp.tile([128, BC], BF16, name="usq")
                nc.vector.tensor_mul(usq, ut, ut)
                nc.tensor.matmul(ssum, ones_col, ut,
                                 start=(mt == 0), stop=(mt == NB_H - 1))
                nc.tensor.matmul(ssq, ones_col, usq,
                                 start=(mt == 0), stop=(mt == NB_H - 1))
        meanB2, rstdB2 = stats_finish(ssum, ssq, H)
        ln_apply(lambda i, u=u_sb: u[:, i:i + 2, :].rearrange("p a b -> p (a b)"),
                 meanB2, rstdB2, gcol, becol,
                 lambda idx, xoff=xoff: x_sb[:, xoff + idx, :],
                 [0, 2, 4, 6])
    s1upool.release()
    s1pool.release()

    # ================= stage 2: block-diagonal dyn layer =================
    dynW = ins["dyn_W"]  # [G, ING, DG]
    y_sb = persist.tile([128, NB_D, BC], BF16, name="y_sb")
    yssum = st_ps.tile([1, BC], F32, name="ssum")
    yssq = st_ps.tile([1, BC], F32, name="ssq")
    KT2 = ING // 128  # 28

    def rhs2(g, k):
        return deterT_sb[:, g * 4 + k, :] if k < 4 else x_sb[:, k - 4, :]

    for g in range(G):
        psums = [mm_tile(m) for m in range(4)]
        for kg in range(7):
            wp = wpool.tile([128, 4, 512], BF16, name="wp")
            nc.sync.dma_start(
                out=wp,
                in_=dynW[g, kg * 512:(kg + 1) * 512, :]
                .rearrange("(kk p) n -> p kk n", p=128),
            )
            for k4 in range(4):
                k = kg * 4 + k4
                for m in range(4):
                    nc.tensor.matmul(
                        psums[m],
                        wp[:, k4, m * 128:(m + 1) * 128],
                        rhs2(g, k),
                        start=(k == 0), stop=(k == KT2 - 1),
                    )
        for m in range(4):
            ft = g * 4 + m
            yt = y_sb[:, ft, :]
            nc.vector.tensor_scalar_add(yt, psums[m],
                                        vecs_sb[:, C_DYNB + ft:C_DYNB + ft + 1])
            ysq = sqp.tile([128, BC], BF16, name="usq")
            nc.vector.tensor_mul(ysq, yt, yt)
            nc.tensor.matmul(yssum, ones_col, yt,
                             start=(ft == 0), stop=(ft == NB_D - 1))
            nc.tensor.matmul(yssq, ones_col, ysq,
                             start=(ft == 0), stop=(ft == NB_D - 1))
    meanB2, rstdB2 = stats_finish(yssum, yssq, D)
    # bulk apply in stage-3 first-use order; Silu only -> one ACT table load
    _border = [0, 2, 5, 3, 6, 1, 4, 7]
    ln_apply(lambda i: y_sb[:, i:i + 2, :].rearrange("p a b -> p (a b)"),
             meanB2, rstdB2, C_GDYN, C_BEDYN, lambda idx: y_sb[:, idx, :],
             [g * 4 + j for g in _border for j in (0, 2)])
    bc_ps.release()
    st_ps.release()
    z2_ps = tc.alloc_tile_pool(name="z2_ps", bufs=1, space="PSUM")
    z_state["z2"] = z2_ps

    # ================= stage 3: GRU gates + output =================
    # zflat f-tiles: reset tj, cand tj+32, update tj+64; block gf = f*128//1536.
    gruW = ins["gru_W"]  # [G, DG, 3*DG]
    zcnt = 0
    for seg in range(8):  # 4 tj per segment; each offset's 4 f-tiles in one block
        tj0 = seg * 4
        tjs = list(range(tj0, tj0 + 4))
        panels = []
        for off in range(3):
            f0 = tj0 * 128 + off * 4096
            gf, col0 = f0 // 1536, f0 % 1536
            wp = wpool.tile([128, 4, 512], BF16, name="wp")
            nc.sync.dma_start(
                out=wp,
                in_=gruW[gf, :, col0:col0 + 512]
                .rearrange("(kk p) n -> p kk n", p=128),
            )
            panels.append((wp, gf))

        def zmm(off, tj):
            nonlocal zcnt
            co = (tj - tj0) * 128
            wp, gf = panels[off]
            zp = z_pair(zcnt)[:, :BC]
            zcnt += 1
            for k in range(4):
                nc.tensor.matmul(zp, wp[:, k, co:co + 128], y_sb[:, gf * 4 + k, :],
                                 start=(k == 0), stop=(k == 3))
            return zp

        r, uu, rc = {}, {}, {}
        for tj in tjs:  # reset gate: sigmoid batch
            zp = zmm(0, tj)
            r[tj] = sqp.tile([128, BC], F32, name=f"r{tj % 2}")
            nc.scalar.activation(r[tj], zp, AF.Sigmoid,
                                 bias=vecs_sb[:, C_GRUB + tj:C_GRUB + tj + 1], scale=1.0)
        for tj in tjs:  # update gate: sigmoid batch
            zp = zmm(2, tj)
            uu[tj] = sqp.tile([128, BC], F32, name=f"uu{tj % 2}")
            nc.scalar.activation(uu[tj], zp, AF.Sigmoid,
                                 bias=vecs_sb[:, C_GRUB + 64 + tj:C_GRUB + 64 + tj + 1], scale=1.0)
        for tj in tjs:  # cand pre-act: DVE + gpsimd only
            zp = zmm(1, tj)
            cp = sqp.tile([128, BC], F32, name="cp")
            nc.vector.tensor_scalar_add(cp, zp,
                                        vecs_sb[:, C_GRUB + 32 + tj:C_GRUB + 32 + tj + 1])
            rc[tj] = sqp.tile([128, BC], F32, name=f"rc{tj % 2}")
            nc.gpsimd.tensor_mul(rc[tj], r[tj], cp)
        for tj in tjs:  # tanh via 2*sigmoid(2x)-1; out = uu*(c-d)+d
            ss = sqp.tile([128, BC], F32, name="cc")
            nc.scalar.activation(ss, rc[tj], AF.Sigmoid, bias=0.0, scale=2.0)
            dt_ = deterT_sb[:, tj, :]
            cd = sqp.tile([128, BC], F32, name="cd")
            nc.vector.scalar_tensor_tensor(cd, ss, 2.0, dt_, OP.mult, OP.subtract)
            o = sqp.tile([128, BC], F32, name="o")
            nc.vector.scalar_tensor_tensor(o, cd, 1.0, uu[tj], OP.subtract, OP.mult)
            nc.gpsimd.tensor_add(o, o, dt_)
            nc.sync.dma_start(out=outT[tj * 128:(tj + 1) * 128, :], in_=o)
    z2_ps.release()
    wpool.release()


_CACHE = {}


def _build():
    if "nc" in _CACHE:
        return _CACHE["nc"]
    nc = bacc.Bacc("TRN2", target_bir_lowering=False, debug=False,
                   num_devices=NCORES)
    ins = {}
    for name, shape, dt in [
        ("deterT", [D, BC], BF16), ("stochT", [S, BC], BF16),
        ("actionT", [A, BC], BF16),
        ("W_d", [D, H], BF16), ("W_s", [S, H], BF16), ("W_a", [A, H], BF16),
        ("dyn_W", [G, ING, DG], BF16), ("gru_W", [G, DG, 3 * DG], BF16),
        ("vecs", [128, NV], F32),
    ]:
        ins[name] = nc.dram_tensor(name, shape, dt, kind="ExternalInput").ap()
    outT = nc.dram_tensor("outT", [D, BC], F32, kind="ExternalOutput").ap()
    with tile.TileContext(nc) as tc:
        _emit(tc, ins, outT)
    nc.compile()
    _CACHE["nc"] = nc
    return nc


def _col_tile(v):
    """[L] -> [128, L//128] with col t holding v[t*128 + p]."""
    return np.ascontiguousarray(v.reshape(-1, 128).T.astype(np.float32))


def _make_vecs(b_d, g_d, be_d, b_s, g_s, be_s, b_a, g_a, be_a,
               dyn_b, g_dyn, be_dyn, gru_b):
    gru_adj = np.array(gru_b, dtype=np.float32).copy()
    gru_adj[2 * D:] -= 1.0
    cols = [b_d, g_d, be_d, b_s, g_s, be_s, b_a, g_a, be_a,
            dyn_b, g_dyn, be_dyn, gru_adj]
    return np.concatenate([_col_tile(np.asarray(c)) for c in cols], axis=1), gru_adj


def kernel(deter, stoch, action,
           W_d, b_d, g_d, be_d,
           W_s, b_s, g_s, be_s,
           W_a, b_a, g_a, be_a,
           dyn_W, dyn_b, g_dyn, be_dyn,
           gru_W, gru_b):
    nc = _build()

    import ml_dtypes
    bf16 = ml_dtypes.bfloat16
    deterT = np.asarray(deter, dtype=np.float32).T.astype(bf16)
    stochT = np.asarray(stoch, dtype=np.float32).T.astype(bf16)
    actionT = np.asarray(action, dtype=np.float32).T.astype(bf16)
    vecs, gru_adj = _make_vecs(b_d, g_d, be_d, b_s, g_s, be_s, b_a, g_a, be_a,
                               dyn_b, g_dyn, be_dyn, gru_b)
    shared = {
        "W_d": np.ascontiguousarray(np.asarray(W_d).astype(bf16)),
        "W_s": np.ascontiguousarray(np.asarray(W_s).astype(bf16)),
        "W_a": np.ascontiguousarray(np.asarray(W_a).astype(bf16)),
        "dyn_W": np.ascontiguousarray(np.asarray(dyn_W).astype(bf16)),
        "gru_W": np.ascontiguousarray(np.asarray(gru_W).astype(bf16)),
        "vecs": vecs,
    }
    in_maps = []
    for c in range(NCORES):
        sl = slice(c * BC, (c + 1) * BC)
        m = dict(shared)
        m["deterT"] = np.ascontiguousarray(deterT[:, sl])
        m["stochT"] = np.ascontiguousarray(stochT[:, sl])
        m["actionT"] = np.ascontiguousarray(actionT[:, sl])
        in_maps.append(m)

    import os
    kw = {}
    if os.environ.get("BASS_TMPDIR"):
        kw["tmpdir"] = os.environ["BASS_TMPDIR"]
    res = run_bass_kernel_spmd(nc, in_maps, list(range(NCORES)), **kw)
    global LAST_RES
    LAST_RES = res
    outT = np.concatenate([res.results[c]["outT"] for c in range(NCORES)], axis=1)
    return np.ascontiguousarray(outT.T)


LAST_RES = None

